# revision 23
# baseline (speedup 1.0000x reference)
"""Causal single-head attention block on 8 TRN2 NeuronCores.

Reference: Q=x@Wq, K=x@Wk, V=x@Wv; S=Q@K^T (no pre-softmax scaling);
causal mask; P=softmax(S); out=(P@V)/sqrt(64).
Shapes: x [4, 2048, 1024] f32, W* [1024, 64] f32 -> out [4, 2048, 64].

Sharding: 8 cores = 4 batches x 2 interleaved query-tile sets.
Core (b, j) handles global 128-row query tiles {2i+j : i=0..7}.

Key design points (vs naive):
  * x is transposed and cast to fp16 on the host; the device loads xT
    directly (no on-chip transposes of x, no duplicate xq load).
  * For SPMD uniformity, j=1 cores get adjacent 128-column blocks of xT
    swapped so query columns sit at even block positions for all cores.
    Key order within a chunk changes, which is harmless (attention sums
    over keys); the causal mask data (per-core) accounts for it.
  * Attention computed transposed: St[t,q] = K @ Q^T per 128-key block,
    so exp() output E already has keys on partitions -> AV matmul needs
    no transposes at all. Rowsum obtained for free via an extra ones
    column appended to V-natural (col 64), accumulated in the same PSUM.
  * Causal mask is preloaded into PSUM with an identity matmul (PE),
    covering the last two 128-key blocks of the diagonal chunk; block
    counts are 2 (even tiles) / 4 (odd tiles) for every core.
  * fp16 for x/W/Q/K/S path, bf16 for E/V (exp range needs bf16);
    1/sqrt(64)=0.125 folded into Wv on the host. rel_err ~5e-3.
  * Input DMAs are merged into 6 large transfers (wkv; rest-of-weights;
    4 key-chunk loads of [128, 8, 512]) to amortize DGE issue overhead.
  * ~30 dummy PE transposes at t=0 warm the PE p-state during the
    initial DMA window so real matmuls run at full clock.
"""

import sys

import numpy as np

try:  # concourse ships in the TRN container; fall back to its known path
    import concourse  # noqa: F401
except ImportError:
    sys.path.insert(0, "/opt/trn_rl_repo")

B, T, C, DK = 4, 2048, 1024, 64
NCH = [1, 1, 2, 2, 3, 3, 4, 4]   # 512-key chunks per local q-tile (both j)
NDUMMY = 30                       # PE p-state warmup transposes
NEG = -30000.0                    # fp16-safe mask value

_CACHE = {}


def _build():
    import concourse.bacc as bacc
    import concourse.tile as tile
    import concourse.mybir as mybir

    f32 = mybir.dt.float32
    f16 = mybir.dt.float16
    bf16 = mybir.dt.bfloat16
    Exp = mybir.ActivationFunctionType.Exp
    Copy = mybir.ActivationFunctionType.Copy

    nc = bacc.Bacc("TRN2", target_bir_lowering=False, debug=False,
                   enable_asserts=False, num_devices=8)

    xt_d = nc.dram_tensor("xt", [8, 128, T], f16, kind="ExternalInput").ap()
    wkv_d = nc.dram_tensor("wkv", [128, 1024], f16, kind="ExternalInput").ap()
    w2_d = nc.dram_tensor("w2", [128, 896], f16, kind="ExternalInput").ap()
    y_d = nc.dram_tensor("y", [128, 512], f32, kind="ExternalOutput").ap()

    with tile.TileContext(nc) as tc:
        with (
            tc.tile_pool(name="persist", bufs=1) as pp,
            tc.tile_pool(name="epool", bufs=5) as ep,
            tc.tile_pool(name="small", bufs=2) as smp,
            tc.tile_pool(name="pa", bufs=2, space="PSUM") as pa,
            tc.tile_pool(name="pb", bufs=2, space="PSUM") as pb,
        ):
            warm = pp.tile([128, 128], f16, tag="warm", name="warm")
            wkv = pp.tile([128, 1024], f16, tag="wkv", name="wkv")
            w2 = pp.tile([128, 896], f16, tag="w2", name="w2")
            wq = w2[:, 0:512]
            dmask = w2[:, 512:768]
            ident = w2[:, 768:896]
            xt = pp.tile([128, 8 * T], f16, tag="xt", name="xt")
            xt3 = xt.rearrange("p (c t) -> p c t", c=8)
            xt5 = xt.rearrange("p (c t4 two par tb) -> p c t4 two par tb",
                               c=8, t4=4, two=2, par=2, tb=128)
            ktvt = [pp.tile([128, 512], f16, tag=f"ktvt{t}", name=f"ktvt{t}")
                    for t in range(4)]
            QT = pp.tile([64, 1024], f16, tag="qt", name="qt")
            vnat = [pp.tile([128, 260], bf16, tag=f"vnat{t}", name=f"vnat{t}")
                    for t in range(4)]
            vnat3 = [v.rearrange("p (k c) -> p k c", k=4) for v in vnat]
            yt = pp.tile([128, 512], f32, tag="yt", name="yt")

            # ---- PE p-state warmup: garbage matmuls during DMA window ----
            nc.vector.memset(warm, 0.0)
            for d in range(NDUMMY):
                ps = pb.tile([128, 1024], f32, tag="st", name="st")
                nc.tensor.matmul(ps[:, 0:128], warm, warm, start=True, stop=True)

            # vnat ones-columns (col 64 of each 65-wide block)
            for t in range(4):
                nc.vector.memset(vnat[t], 1.0)

            # ---- input DMAs: xt in 256-key half-slabs (first in quarters) ----
            nc.sync.dma_start(wkv, wkv_d)
            xt_dr = xt_d.rearrange("c p t -> p c t")
            nc.sync.dma_start(xt3[:, :, 0:128], xt_dr[:, :, 0:128])
            nc.sync.dma_start(xt3[:, :, 128:256], xt_dr[:, :, 128:256])
            nc.sync.dma_start(w2, w2_d)
            for hh in range(1, 8):
                nc.sync.dma_start(
                    xt3[:, :, 256 * hh:256 * (hh + 1)],
                    xt_dr[:, :, 256 * hh:256 * (hh + 1)],
                )

            def st_group(i, nch, nbd, cs):
                """St matmuls for one pair-group of q-tile i; returns
                (E tile, [(col_off, chunk, k)]) after the fused exp."""
                sps = pb.tile([128, 1024], f32, tag="st", name="st")
                col = 0
                blocks = []
                for c in cs:
                    nb = nbd if c == nch - 1 else 4
                    if c == nch - 1:
                        for k in range(nb - 2):
                            nc.tensor.matmul(
                                sps[:, col + 128 * k:col + 128 * (k + 1)],
                                ktvt[c][0:64, 128 * k:128 * (k + 1)],
                                QT[:, 128 * i:128 * (i + 1)],
                                start=True, stop=True,
                            )
                        nc.tensor.matmul(
                            sps[:, col + 128 * (nb - 2):col + 128 * nb],
                            ident, dmask,
                            start=True, stop=False,
                        )
                        for k in (nb - 2, nb - 1):
                            nc.tensor.matmul(
                                sps[:, col + 128 * k:col + 128 * (k + 1)],
                                ktvt[c][0:64, 128 * k:128 * (k + 1)],
                                QT[:, 128 * i:128 * (i + 1)],
                                start=False, stop=(k == nb - 1),
                                skip_group_check=True,
                            )
                    else:
                        for k in range(4):
                            nc.tensor.matmul(
                                sps[:, col + 128 * k:col + 128 * (k + 1)],
                                ktvt[c][0:64, 128 * k:128 * (k + 1)],
                                QT[:, 128 * i:128 * (i + 1)],
                                start=True, stop=True,
                            )
                    blocks += [(col + 128 * k, c, k) for k in range(nb)]
                    col += 128 * nb
                E = ep.tile([128, 1024], bf16, tag="E", name="E")
                nc.scalar.activation(E[:, 0:col], sps[:, 0:col], Exp)
                return E, blocks

            def av_group(po, E, blocks, m, nmm):
                for off, c, k in blocks:
                    nc.tensor.matmul(
                        po[:, 0:65],
                        E[:, off:off + 128],
                        vnat3[c][:, k, :],
                        start=(m == 0), stop=(m == nmm - 1),
                    )
                    m += 1
                return m

            def finalize(i, po):
                rinv = smp.tile([128, 1], f32, tag="rinv", name="rinv")
                nc.vector.reciprocal(rinv, po[:, 64:65])
                nc.vector.tensor_scalar_mul(yt[:, 64 * i:64 * (i + 1)],
                                            po[:, 0:64], rinv[:, 0:1])
                if i % 2 == 1:
                    tch = i // 2
                    nc.sync.dma_start(y_d[:, 128 * tch:128 * (tch + 1)],
                                      yt[:, 128 * tch:128 * (tch + 1)])

            # Software-pipelined half-slab loop. Per half h (q-tile i=h):
            #   Q proj; KV proj; St+exp of old groups; deferred AV+finalize
            #   of tile h-1's diagonal; V-natural; St+exp of diagonal group;
            #   AV of old groups. The diagonal AV runs in half h+1 so its
            #   exp latency hides under h+1's projections.
            prev = None   # (i, po, E, blocks, m, nmm) of previous half's diag
            for h in range(8):
                tch, half = divmod(h, 2)
                i = h
                hoff = 256 * h
                nch = NCH[i]
                nbd = 2 + 2 * (i % 2)
                if i == 7:  # short diagonal group -> shorter kernel tail
                    groups = [(0, 1), (2,), (3,)]
                else:
                    groups = [tuple(c for c in (g, g + 1) if c < nch)
                              for g in range(0, nch, 2)]
                nmm = 4 * (nch - 1) + nbd

                ps = pa.tile([128, 512], f32, tag="pa", name="ps")
                po = ps[:, 384:449]
                # Q projection: this half's q-block -> QT cols of tile i
                for cj in range(8):
                    nc.tensor.matmul(
                        ps[0:64, 256:384],
                        wq[:, 64 * cj:64 * (cj + 1)],
                        xt5[:, cj, tch, half, 0, :],
                        start=(cj == 0), stop=(cj == 7),
                    )
                nc.vector.tensor_copy(QT[:, 128 * i:128 * (i + 1)],
                                      ps[0:64, 256:384])
                # KV projection for this half (h=0: two quarter passes)
                for q0, q1 in ([(0, 128), (128, 256)] if h == 0 else [(0, 256)]):
                    for cj in range(8):
                        nc.tensor.matmul(
                            ps[:, q0:q1],
                            wkv[:, 128 * cj:128 * (cj + 1)],
                            xt3[:, cj, hoff + q0:hoff + q1],
                            start=(cj == 0), stop=(cj == 7),
                        )
                nc.vector.tensor_copy(
                    ktvt[tch][:, 256 * half:256 * (half + 1)], ps[:, 0:256])
                # St + exp for groups needing only chunks < tch
                done = [st_group(i, nch, nbd, cs)
                        for cs in groups if max(cs) < tch]
                # deferred diagonal AV + finalize of the previous tile
                if prev is not None:
                    pi, ppo, pE, pbl, pm, pnmm = prev
                    av_group(ppo, pE, pbl, pm, pnmm)
                    finalize(pi, ppo)
                # V natural for this half's two key blocks
                vps = pa.tile([128, 512], f16, tag="pav", name="vps", bufs=1)
                for kk in range(2):
                    nc.tensor.transpose(
                        vps[:, 64 * kk:64 * (kk + 1)],
                        ktvt[tch][64:128,
                                  hoff % 512 + 128 * kk:hoff % 512 + 128 * (kk + 1)],
                        ident[64:128, 64:128],
                    )
                vps3 = vps.rearrange("p (k c) -> p k c", k=8)
                nc.vector.tensor_copy(
                    vnat3[tch][:, 2 * half:2 * half + 2, 0:64], vps3[:, 0:2, :])
                # diagonal group St + exp
                dE, dbl = next(st_group(i, nch, nbd, cs)
                               for cs in groups if max(cs) == tch)
                # AV of old groups (their exps are long done)
                m = 0
                for E, bl in done:
                    m = av_group(po, E, bl, m, nmm)
                prev = (i, po, dE, dbl, m, nmm)
            pi, ppo, pE, pbl, pm, pnmm = prev
            av_group(ppo, pE, pbl, pm, pnmm)
            finalize(pi, ppo)

    nc.compile()
    return nc


def _host_inputs(x, Wq, Wk, Wv):
    """Per-core input maps. Core c = 2*b + j."""
    f16 = np.float16
    wkv = np.empty((128, 1024), f16)
    Wv8 = Wv * 0.125
    for cj in range(8):
        wkv[:, 128 * cj:128 * cj + 64] = Wk[128 * cj:128 * (cj + 1), :]
        wkv[:, 128 * cj + 64:128 * (cj + 1)] = Wv8[128 * cj:128 * (cj + 1), :]
    wq = np.empty((128, 512), f16)
    for cj in range(8):
        wq[:, 64 * cj:64 * (cj + 1)] = Wq[128 * cj:128 * (cj + 1), :]
    tri = np.zeros((128, 128), np.float32)
    tri[np.arange(128)[:, None] > np.arange(128)[None, :]] = NEG
    w2 = [np.zeros((128, 896), f16) for _ in range(2)]
    for j in range(2):
        w2[j][:, 0:512] = wq
        w2[j][:, 512:640] = tri          # diag block of preload pair
        w2[j][:, 640:768] = NEG if j == 0 else 0.0  # past-diag block
        w2[j][:, 768:896] = np.eye(128, dtype=f16)

    in_maps = []
    for core in range(8):
        b, j = divmod(core, 2)
        xT = x[b].T.astype(f16)          # [1024, 2048]
        if j == 1:
            # swap adjacent 128-col blocks so q-cols sit at even positions
            xT = xT.reshape(1024, 8, 2, 128)[:, :, ::-1, :].reshape(1024, 2048)
        in_maps.append({
            "xt": np.ascontiguousarray(xT).reshape(8, 128, T),
            "wkv": wkv,
            "w2": w2[j],
        })
    return in_maps


def kernel(x, Wq, Wk, Wv):
    from concourse.bass_utils import run_bass_kernel_spmd

    x = np.asarray(x, dtype=np.float32)
    Wq = np.asarray(Wq, dtype=np.float32)
    Wk = np.asarray(Wk, dtype=np.float32)
    Wv = np.asarray(Wv, dtype=np.float32)

    if "nc" not in _CACHE:
        _CACHE["nc"] = _build()
    nc = _CACHE["nc"]

    in_maps = _host_inputs(x, Wq, Wk, Wv)
    res = run_bass_kernel_spmd(nc, in_maps, core_ids=list(range(8)))
    out = np.empty((B, T, DK), dtype=np.float32)
    for core in range(8):
        b, j = divmod(core, 2)
        yloc = res.results[core]["y"]    # [128, 512]
        for i in range(8):
            g = 2 * i + j
            out[b, 128 * g:128 * (g + 1), :] = yloc[:, 64 * i:64 * (i + 1)]
    return out


# revision 26
# speedup vs baseline: 1.0674x; 1.0674x over previous
"""Causal single-head attention block on 8 TRN2 NeuronCores.

Reference: Q=x@Wq, K=x@Wk, V=x@Wv; S=Q@K^T (no pre-softmax scaling);
causal mask; P=softmax(S); out=(P@V)/sqrt(64).
Shapes: x [4, 2048, 1024] f32, W* [1024, 64] f32 -> out [4, 2048, 64].

Sharding: 8 cores = 4 batches x 2 interleaved query-tile sets.
Core (b, j) handles global 128-row query tiles {2i+j : i=0..7}.

Key design points (vs naive):
  * x is transposed and cast to fp16 on the host; the device loads xT
    directly (no on-chip transposes of x, no duplicate xq load).
  * For SPMD uniformity, j=1 cores get adjacent 128-column blocks of xT
    swapped so query columns sit at even block positions for all cores.
    Key order within a chunk changes, which is harmless (attention sums
    over keys); the causal mask data (per-core) accounts for it.
  * Attention computed transposed: St[t,q] = K @ Q^T per 128-key block,
    so exp() output E already has keys on partitions -> AV matmul needs
    no transposes at all. Rowsum obtained for free via an extra ones
    column appended to V-natural (col 64), accumulated in the same PSUM.
  * Causal mask is preloaded into PSUM with an identity matmul (PE),
    covering the last two 128-key blocks of the diagonal chunk; block
    counts are 2 (even tiles) / 4 (odd tiles) for every core.
  * fp16 for x/W/Q/K/S path, bf16 for E/V (exp range needs bf16);
    1/sqrt(64)=0.125 folded into Wv on the host. rel_err ~5e-3.
  * Input DMAs are merged into 6 large transfers (wkv; rest-of-weights;
    4 key-chunk loads of [128, 8, 512]) to amortize DGE issue overhead.
  * ~30 dummy PE transposes at t=0 warm the PE p-state during the
    initial DMA window so real matmuls run at full clock.
"""

import sys

import numpy as np

try:  # concourse ships in the TRN container; fall back to its known path
    import concourse  # noqa: F401
except ImportError:
    sys.path.insert(0, "/opt/trn_rl_repo")

B, T, C, DK = 4, 2048, 1024, 64
NCH = [1, 1, 2, 2, 3, 3, 4, 4]   # 512-key chunks per local q-tile (both j)
NDUMMY = 30                       # PE p-state warmup transposes
NEG = -30000.0                    # fp16-safe mask value

_CACHE = {}


def _build():
    import concourse.bacc as bacc
    import concourse.tile as tile
    import concourse.mybir as mybir

    f32 = mybir.dt.float32
    f16 = mybir.dt.float16
    bf16 = mybir.dt.bfloat16
    Exp = mybir.ActivationFunctionType.Exp
    Copy = mybir.ActivationFunctionType.Copy

    nc = bacc.Bacc("TRN2", target_bir_lowering=False, debug=False,
                   enable_asserts=False, num_devices=8)

    xt_d = nc.dram_tensor("xt", [8, 128, T], f16, kind="ExternalInput").ap()
    wkv_d = nc.dram_tensor("wkv", [128, 1024], f16, kind="ExternalInput").ap()
    w2_d = nc.dram_tensor("w2", [128, 896], f16, kind="ExternalInput").ap()
    y_d = nc.dram_tensor("y", [128, 512], f32, kind="ExternalOutput").ap()

    with tile.TileContext(nc) as tc:
        with (
            tc.tile_pool(name="persist", bufs=1) as pp,
            tc.tile_pool(name="epool", bufs=5) as ep,
            tc.tile_pool(name="small", bufs=2) as smp,
            tc.tile_pool(name="pa", bufs=2, space="PSUM") as pa,
            tc.tile_pool(name="pb", bufs=2, space="PSUM") as pb,
        ):
            warm = pp.tile([128, 128], f16, tag="warm", name="warm")
            wkv = pp.tile([128, 1024], f16, tag="wkv", name="wkv")
            w2 = pp.tile([128, 896], f16, tag="w2", name="w2")
            wq = w2[:, 0:512]
            dmask = w2[:, 512:768]
            ident = w2[:, 768:896]
            xt = pp.tile([128, 8 * T], f16, tag="xt", name="xt")
            xt3 = xt.rearrange("p (c t) -> p c t", c=8)
            xt5 = xt.rearrange("p (c t4 two par tb) -> p c t4 two par tb",
                               c=8, t4=4, two=2, par=2, tb=128)
            ktvt = [pp.tile([128, 512], f16, tag=f"ktvt{t}", name=f"ktvt{t}")
                    for t in range(4)]
            QT = pp.tile([64, 1024], f16, tag="qt", name="qt")
            vnat = [pp.tile([128, 260], bf16, tag=f"vnat{t}", name=f"vnat{t}")
                    for t in range(4)]
            vnat3 = [v.rearrange("p (k c) -> p k c", k=4) for v in vnat]
            yt = pp.tile([128, 512], f32, tag="yt", name="yt")

            # ---- PE p-state warmup: garbage matmuls during DMA window ----
            nc.vector.memset(warm, 0.0)
            for d in range(NDUMMY):
                ps = pb.tile([128, 1024], f32, tag="st", name="st")
                nc.tensor.matmul(ps[:, 0:128], warm, warm, start=True, stop=True)

            # vnat ones-columns (col 64 of each 65-wide block)
            for t in range(4):
                nc.vector.memset(vnat[t], 1.0)

            # ---- input DMAs: xt in 256-key half-slabs (first in quarters) ----
            nc.sync.dma_start(wkv, wkv_d)
            xt_dr = xt_d.rearrange("c p t -> p c t")
            nc.sync.dma_start(xt3[:, :, 0:128], xt_dr[:, :, 0:128])
            nc.sync.dma_start(xt3[:, :, 128:256], xt_dr[:, :, 128:256])
            nc.sync.dma_start(w2, w2_d)
            for hh in range(1, 8):
                nc.sync.dma_start(
                    xt3[:, :, 256 * hh:256 * (hh + 1)],
                    xt_dr[:, :, 256 * hh:256 * (hh + 1)],
                )

            def st_group(i, nch, nbd, cs):
                """St matmuls for one pair-group of q-tile i; returns
                (sps, E tile, [(col_off, chunk, k)]) after the fused exp."""
                sps = pb.tile([128, 1024], f32, tag="st", name="st")
                col = 0
                blocks = []
                for c in cs:
                    nb = nbd if c == nch - 1 else 4
                    if c == nch - 1:
                        for k in range(nb - 2):
                            nc.tensor.matmul(
                                sps[:, col + 128 * k:col + 128 * (k + 1)],
                                ktvt[c][0:64, 128 * k:128 * (k + 1)],
                                QT[:, 128 * i:128 * (i + 1)],
                                start=True, stop=True,
                            )
                        nc.tensor.matmul(
                            sps[:, col + 128 * (nb - 2):col + 128 * nb],
                            ident, dmask,
                            start=True, stop=False,
                        )
                        for k in (nb - 2, nb - 1):
                            nc.tensor.matmul(
                                sps[:, col + 128 * k:col + 128 * (k + 1)],
                                ktvt[c][0:64, 128 * k:128 * (k + 1)],
                                QT[:, 128 * i:128 * (i + 1)],
                                start=False, stop=(k == nb - 1),
                                skip_group_check=True,
                            )
                    else:
                        for k in range(4):
                            nc.tensor.matmul(
                                sps[:, col + 128 * k:col + 128 * (k + 1)],
                                ktvt[c][0:64, 128 * k:128 * (k + 1)],
                                QT[:, 128 * i:128 * (i + 1)],
                                start=True, stop=True,
                            )
                    blocks += [(col + 128 * k, c, k) for k in range(nb)]
                    col += 128 * nb
                E = ep.tile([128, 1024], bf16, tag="E", name="E")
                nc.scalar.activation(E[:, 0:col], sps[:, 0:col], Exp)
                return sps, E, blocks

            def av_group(po, E, blocks, m, nmm):
                for off, c, k in blocks:
                    nc.tensor.matmul(
                        po[:, 0:65],
                        E[:, off:off + 128],
                        vnat3[c][:, k, :],
                        start=(m == 0), stop=(m == nmm - 1),
                    )
                    m += 1
                return m

            def finalize(i, po):
                rinv = smp.tile([128, 1], f32, tag="rinv", name="rinv")
                nc.vector.reciprocal(rinv, po[:, 64:65])
                nc.vector.tensor_scalar_mul(yt[:, 64 * i:64 * (i + 1)],
                                            po[:, 0:64], rinv[:, 0:1])
                if i % 2 == 1:
                    tch = i // 2
                    nc.sync.dma_start(y_d[:, 128 * tch:128 * (tch + 1)],
                                      yt[:, 128 * tch:128 * (tch + 1)])

            # Software-pipelined half-slab loop. Per half h (q-tile i=h):
            #   KV proj; Q proj; St+exp of old groups; deferred AV+finalize
            #   of tile h-1's diagonal; V-natural; St+exp of diagonal group;
            #   AV of old groups. The diagonal AV runs in half h+1 so its
            #   exp latency hides under h+1's projections. po lives in the
            #   diagonal St tile (cols 896:961 — diag groups are <=768 wide).
            prev = None   # (i, po, E, blocks, m, nmm) of previous half's diag
            for h in range(8):
                tch, half = divmod(h, 2)
                i = h
                hoff = 256 * h
                nch = NCH[i]
                nbd = 2 + 2 * (i % 2)
                if i in (3, 7):  # split pairs: diag group stays narrow
                    groups = [(c,) for c in range(nch - 2)] + \
                             [(nch - 2,), (nch - 1,)]
                    if nch == 4:
                        groups = [(0, 1), (2,), (3,)]
                else:
                    groups = [tuple(c for c in (g, g + 1) if c < nch)
                              for g in range(0, nch, 2)]
                nmm = 4 * (nch - 1) + nbd

                ps = pa.tile([128, 512], f32, tag="pa", name="ps")
                # KV projection for this half (h=0: two quarter passes)
                for q0, q1 in ([(0, 128), (128, 256)] if h == 0 else [(0, 256)]):
                    for cj in range(8):
                        nc.tensor.matmul(
                            ps[:, q0:q1],
                            wkv[:, 128 * cj:128 * (cj + 1)],
                            xt3[:, cj, hoff + q0:hoff + q1],
                            start=(cj == 0), stop=(cj == 7),
                        )
                nc.vector.tensor_copy(
                    ktvt[tch][:, 256 * half:256 * (half + 1)], ps[:, 0:256])
                # Q projection: this half's q-block -> QT cols of tile i
                for cj in range(8):
                    nc.tensor.matmul(
                        ps[0:64, 256:384],
                        wq[:, 64 * cj:64 * (cj + 1)],
                        xt5[:, cj, tch, half, 0, :],
                        start=(cj == 0), stop=(cj == 7),
                    )
                nc.vector.tensor_copy(QT[:, 128 * i:128 * (i + 1)],
                                      ps[0:64, 256:384])
                # St + exp for groups needing only chunks < tch
                done = [st_group(i, nch, nbd, cs)
                        for cs in groups if max(cs) < tch]
                # deferred diagonal AV + finalize of the previous tile
                if prev is not None:
                    pi, ppo, pE, pbl, pm, pnmm = prev
                    av_group(ppo, pE, pbl, pm, pnmm)
                    finalize(pi, ppo)
                # V natural for this half's two key blocks
                vps = pa.tile([128, 512], f16, tag="pav", name="vps", bufs=1)
                for kk in range(2):
                    nc.tensor.transpose(
                        vps[:, 64 * kk:64 * (kk + 1)],
                        ktvt[tch][64:128,
                                  hoff % 512 + 128 * kk:hoff % 512 + 128 * (kk + 1)],
                        ident[64:128, 64:128],
                    )
                vps3 = vps.rearrange("p (k c) -> p k c", k=8)
                nc.vector.tensor_copy(
                    vnat3[tch][:, 2 * half:2 * half + 2, 0:64], vps3[:, 0:2, :])
                # diagonal group St + exp; po in its tail columns
                dsps, dE, dbl = next(st_group(i, nch, nbd, cs)
                                     for cs in groups if max(cs) == tch)
                po = dsps[:, 896:961]
                # AV of old groups (their exps are long done)
                m = 0
                for _, E, bl in done:
                    m = av_group(po, E, bl, m, nmm)
                prev = (i, po, dE, dbl, m, nmm)
            pi, ppo, pE, pbl, pm, pnmm = prev
            av_group(ppo, pE, pbl, pm, pnmm)
            finalize(pi, ppo)

    nc.compile()
    return nc


def _host_inputs(x, Wq, Wk, Wv):
    """Per-core input maps. Core c = 2*b + j."""
    f16 = np.float16
    wkv = np.empty((128, 1024), f16)
    Wv8 = Wv * 0.125
    for cj in range(8):
        wkv[:, 128 * cj:128 * cj + 64] = Wk[128 * cj:128 * (cj + 1), :]
        wkv[:, 128 * cj + 64:128 * (cj + 1)] = Wv8[128 * cj:128 * (cj + 1), :]
    wq = np.empty((128, 512), f16)
    for cj in range(8):
        wq[:, 64 * cj:64 * (cj + 1)] = Wq[128 * cj:128 * (cj + 1), :]
    tri = np.zeros((128, 128), np.float32)
    tri[np.arange(128)[:, None] > np.arange(128)[None, :]] = NEG
    w2 = [np.zeros((128, 896), f16) for _ in range(2)]
    for j in range(2):
        w2[j][:, 0:512] = wq
        w2[j][:, 512:640] = tri          # diag block of preload pair
        w2[j][:, 640:768] = NEG if j == 0 else 0.0  # past-diag block
        w2[j][:, 768:896] = np.eye(128, dtype=f16)

    in_maps = []
    for core in range(8):
        b, j = divmod(core, 2)
        xT = x[b].T.astype(f16)          # [1024, 2048]
        if j == 1:
            # swap adjacent 128-col blocks so q-cols sit at even positions
            xT = xT.reshape(1024, 8, 2, 128)[:, :, ::-1, :].reshape(1024, 2048)
        in_maps.append({
            "xt": np.ascontiguousarray(xT).reshape(8, 128, T),
            "wkv": wkv,
            "w2": w2[j],
        })
    return in_maps


def kernel(x, Wq, Wk, Wv):
    from concourse.bass_utils import run_bass_kernel_spmd

    x = np.asarray(x, dtype=np.float32)
    Wq = np.asarray(Wq, dtype=np.float32)
    Wk = np.asarray(Wk, dtype=np.float32)
    Wv = np.asarray(Wv, dtype=np.float32)

    if "nc" not in _CACHE:
        _CACHE["nc"] = _build()
    nc = _CACHE["nc"]

    in_maps = _host_inputs(x, Wq, Wk, Wv)
    res = run_bass_kernel_spmd(nc, in_maps, core_ids=list(range(8)))
    out = np.empty((B, T, DK), dtype=np.float32)
    for core in range(8):
        b, j = divmod(core, 2)
        yloc = res.results[core]["y"]    # [128, 512]
        for i in range(8):
            g = 2 * i + j
            out[b, 128 * g:128 * (g + 1), :] = yloc[:, 64 * i:64 * (i + 1)]
    return out


# revision 27
# speedup vs baseline: 1.1452x; 1.0729x over previous
"""Causal single-head attention block on 8 TRN2 NeuronCores.

Reference: Q=x@Wq, K=x@Wk, V=x@Wv; S=Q@K^T (no pre-softmax scaling);
causal mask; P=softmax(S); out=(P@V)/sqrt(64).
Shapes: x [4, 2048, 1024] f32, W* [1024, 64] f32 -> out [4, 2048, 64].

Sharding: 8 cores = 4 batches x 2 interleaved query-tile sets.
Core (b, j) handles global 128-row query tiles {2i+j : i=0..7}.

Key design points (vs naive):
  * x is transposed and cast to fp16 on the host; the device loads xT
    directly (no on-chip transposes of x, no duplicate xq load).
  * For SPMD uniformity, j=1 cores get adjacent 128-column blocks of xT
    swapped so query columns sit at even block positions for all cores.
    Key order within a chunk changes, which is harmless (attention sums
    over keys); the causal mask data (per-core) accounts for it.
  * Attention computed transposed: St[t,q] = K @ Q^T per 128-key block,
    so exp() output E already has keys on partitions -> AV matmul needs
    no transposes at all. Rowsum obtained for free via an extra ones
    column appended to V-natural (col 64), accumulated in the same PSUM.
  * Causal mask is preloaded into PSUM with an identity matmul (PE),
    covering the last two 128-key blocks of the diagonal chunk; block
    counts are 2 (even tiles) / 4 (odd tiles) for every core.
  * fp16 for x/W/Q/K/S path, bf16 for E/V (exp range needs bf16);
    1/sqrt(64)=0.125 folded into Wv on the host. rel_err ~5e-3.
  * Input DMAs are merged into 6 large transfers (wkv; rest-of-weights;
    4 key-chunk loads of [128, 8, 512]) to amortize DGE issue overhead.
  * ~30 dummy PE transposes at t=0 warm the PE p-state during the
    initial DMA window so real matmuls run at full clock.
"""

import sys

import numpy as np

try:  # concourse ships in the TRN container; fall back to its known path
    import concourse  # noqa: F401
except ImportError:
    sys.path.insert(0, "/opt/trn_rl_repo")

B, T, C, DK = 4, 2048, 1024, 64
NCH = [1, 1, 2, 2, 3, 3, 4, 4]   # 512-key chunks per local q-tile (both j)
NDUMMY = 30                       # PE p-state warmup transposes
NEG = -30000.0                    # fp16-safe mask value

_CACHE = {}


def _build():
    import concourse.bacc as bacc
    import concourse.tile as tile
    import concourse.mybir as mybir

    f32 = mybir.dt.float32
    f16 = mybir.dt.float16
    bf16 = mybir.dt.bfloat16
    Exp = mybir.ActivationFunctionType.Exp
    Copy = mybir.ActivationFunctionType.Copy

    nc = bacc.Bacc("TRN2", target_bir_lowering=False, debug=False,
                   enable_asserts=False, num_devices=8)

    xt_d = nc.dram_tensor("xt", [8, 128, T], f16, kind="ExternalInput").ap()
    wkv_d = nc.dram_tensor("wkv", [128, 1024], f16, kind="ExternalInput").ap()
    w2_d = nc.dram_tensor("w2", [128, 896], f16, kind="ExternalInput").ap()
    y_d = nc.dram_tensor("y", [128, 512], f32, kind="ExternalOutput").ap()

    with tile.TileContext(nc) as tc:
        with (
            tc.tile_pool(name="persist", bufs=1) as pp,
            tc.tile_pool(name="epool", bufs=4) as ep,
            tc.tile_pool(name="small", bufs=2) as smp,
            tc.tile_pool(name="pa", bufs=2, space="PSUM") as pa,
            tc.tile_pool(name="pb", bufs=2, space="PSUM") as pb,
            tc.tile_pool(name="pc", bufs=1, space="PSUM") as pc,
        ):
            warm = pp.tile([128, 128], f16, tag="warm", name="warm")
            wkv = pp.tile([128, 1024], f16, tag="wkv", name="wkv")
            w2 = pp.tile([128, 896], f16, tag="w2", name="w2")
            wq = w2[:, 0:512]
            dmask = w2[:, 512:768]
            ident = w2[:, 768:896]
            xt = pp.tile([128, 8 * T], f16, tag="xt", name="xt")
            xt3 = xt.rearrange("p (c t) -> p c t", c=8)
            xt5 = xt.rearrange("p (c t4 two par tb) -> p c t4 two par tb",
                               c=8, t4=4, two=2, par=2, tb=128)
            ktvt = [pp.tile([128, 512], f16, tag=f"ktvt{t}", name=f"ktvt{t}")
                    for t in range(4)]
            QT = pp.tile([64, 1024], f16, tag="qt", name="qt")
            vnat = [pp.tile([128, 260], bf16, tag=f"vnat{t}", name=f"vnat{t}")
                    for t in range(4)]
            vnat3 = [v.rearrange("p (k c) -> p k c", k=4) for v in vnat]
            yt = pp.tile([128, 512], f32, tag="yt", name="yt")

            # ---- PE p-state warmup: garbage matmuls during DMA window ----
            nc.vector.memset(warm, 0.0)
            for d in range(NDUMMY):
                ps = pa.tile([128, 512], f32, tag="pa", name="kvps")
                nc.tensor.matmul(ps[:, 0:128], warm, warm, start=True, stop=True)

            # vnat ones-columns (col 64 of each 65-wide block)
            for t in range(4):
                nc.vector.memset(vnat[t], 1.0)

            # ---- input DMAs: big merged transfers; chunk 0 split in halves ----
            nc.sync.dma_start(wkv, wkv_d)
            xt_dr = xt_d.rearrange("c p t -> p c t")
            nc.sync.dma_start(xt3[:, :, 0:256], xt_dr[:, :, 0:256])
            nc.sync.dma_start(xt3[:, :, 256:512], xt_dr[:, :, 256:512])
            nc.sync.dma_start(w2, w2_d)
            for tch in range(1, 4):
                nc.sync.dma_start(
                    xt3[:, :, 512 * tch:512 * (tch + 1)],
                    xt_dr[:, :, 512 * tch:512 * (tch + 1)],
                )

            for tch in range(4):
                t0 = 512 * tch
                # ---- fused K|V projection: rows 0:64=K^T, 64:128=V'^T ----
                ps = pa.tile([128, 512], f32, tag="pa", name="kvps")
                if tch == 0:
                    # chunk 0 arrives in two half-DMAs: start PE earlier
                    for h in range(2):
                        for cj in range(8):
                            nc.tensor.matmul(
                                ps[:, 256 * h:256 * (h + 1)],
                                wkv[:, 128 * cj:128 * (cj + 1)],
                                xt3[:, cj, 256 * h:256 * (h + 1)],
                                start=(cj == 0), stop=(cj == 7),
                            )
                else:
                    for cj in range(8):
                        nc.tensor.matmul(
                            ps,
                            wkv[:, 128 * cj:128 * (cj + 1)],
                            xt3[:, cj, t0:t0 + 512],
                            start=(cj == 0), stop=(cj == 7),
                        )
                nc.vector.tensor_copy(ktvt[tch], ps)
                # ---- Q projection for the chunk's two q-tiles ----
                qps = pa.tile([128, 512], f32, tag="pa", name="qps")
                for cj in range(8):
                    nc.tensor.matmul(
                        qps[0:64, 0:256],
                        wq[:, 64 * cj:64 * (cj + 1)],
                        xt5[:, cj, tch, :, 0, :],
                        start=(cj == 0), stop=(cj == 7),
                    )
                if tch == 0:  # ACT is idle early; DVE busy with ktvt copy
                    nc.scalar.activation(QT[:, 0:256], qps[0:64, 0:256], Copy)
                else:         # later ACT is exp-saturated; DVE has slack
                    nc.vector.tensor_copy(QT[:, 256 * tch:256 * (tch + 1)],
                                          qps[0:64, 0:256])
                # ---- V natural [t, v] + ones column ----
                vps = pa.tile([128, 512], f16, tag="pav", name="vps", bufs=1)
                for k in range(4):
                    nc.tensor.transpose(
                        vps[:, 64 * k:64 * (k + 1)],
                        ktvt[tch][64:128, 128 * k:128 * (k + 1)],
                        ident[64:128, 64:128],
                    )
                vps3 = vps.rearrange("p (k c) -> p k c", k=8)
                nc.vector.tensor_copy(vnat3[tch][:, :, 0:64], vps3[:, 0:4, :])

                # ---- attention for q-tiles 2*tch, 2*tch+1 ----
                # Chunks are processed in pairs sharing one [128,1024] PSUM
                # tile and ONE exp per pair (halves ACT per-inst overhead).
                # (last chunk: bigger tile 7 first so the kernel tail is short)
                for dlt in ((1, 0) if tch == 3 else (0, 1)):
                    i = 2 * tch + dlt
                    nch = NCH[i]
                    nbd = 2 + 2 * (i % 2)
                    groups = [tuple(c for c in (g, g + 1) if c < nch)
                              for g in range(0, nch, 2)]
                    Es = []   # (E tile, [(col_off, chunk, k), ...])
                    for cs in groups:
                        sps = pb.tile([128, 1024], f32, tag="st", name="st")
                        col = 0
                        blocks = []
                        for c in cs:
                            if c == nch - 1:
                                for k in range(nbd - 2):
                                    nc.tensor.matmul(
                                        sps[:, col + 128 * k:col + 128 * (k + 1)],
                                        ktvt[c][0:64, 128 * k:128 * (k + 1)],
                                        QT[:, 128 * i:128 * (i + 1)],
                                        start=True, stop=True,
                                    )
                                nc.tensor.matmul(
                                    sps[:, col + 128 * (nbd - 2):col + 128 * nbd],
                                    ident, dmask,
                                    start=True, stop=False,
                                )
                                for k in (nbd - 2, nbd - 1):
                                    nc.tensor.matmul(
                                        sps[:, col + 128 * k:col + 128 * (k + 1)],
                                        ktvt[c][0:64, 128 * k:128 * (k + 1)],
                                        QT[:, 128 * i:128 * (i + 1)],
                                        start=False, stop=(k == nbd - 1),
                                        skip_group_check=True,
                                    )
                                nb = nbd
                            else:
                                nb = 4
                                for k in range(4):
                                    nc.tensor.matmul(
                                        sps[:, col + 128 * k:col + 128 * (k + 1)],
                                        ktvt[c][0:64, 128 * k:128 * (k + 1)],
                                        QT[:, 128 * i:128 * (i + 1)],
                                        start=True, stop=True,
                                    )
                            blocks += [(col + 128 * k, c, k) for k in range(nb)]
                            col += 128 * nb
                        E = ep.tile([128, 1024], bf16, tag="E", name="E")
                        nc.scalar.activation(E[:, 0:col], sps[:, 0:col], Exp)
                        Es.append((E, blocks))
                    po = pc.tile([128, 128], f32, tag="po", name="po")
                    nmm = sum(len(bl) for _, bl in Es)
                    m = 0
                    for E, bl in Es:
                        for off, c, k in bl:
                            nc.tensor.matmul(
                                po[:, 0:65],
                                E[:, off:off + 128],
                                vnat3[c][:, k, :],
                                start=(m == 0), stop=(m == nmm - 1),
                            )
                            m += 1
                    rinv = smp.tile([128, 1], f32, tag="rinv", name="rinv")
                    nc.vector.reciprocal(rinv, po[:, 64:65])
                    nc.vector.tensor_scalar_mul(yt[:, 64 * i:64 * (i + 1)],
                                                po[:, 0:64], rinv[:, 0:1])
                nc.sync.dma_start(y_d[:, 128 * tch:128 * (tch + 1)],
                                  yt[:, 128 * tch:128 * (tch + 1)])

    nc.compile()
    return nc


def _host_inputs(x, Wq, Wk, Wv):
    """Per-core input maps. Core c = 2*b + j."""
    f16 = np.float16
    wkv = np.empty((128, 1024), f16)
    Wv8 = Wv * 0.125
    for cj in range(8):
        wkv[:, 128 * cj:128 * cj + 64] = Wk[128 * cj:128 * (cj + 1), :]
        wkv[:, 128 * cj + 64:128 * (cj + 1)] = Wv8[128 * cj:128 * (cj + 1), :]
    wq = np.empty((128, 512), f16)
    for cj in range(8):
        wq[:, 64 * cj:64 * (cj + 1)] = Wq[128 * cj:128 * (cj + 1), :]
    tri = np.zeros((128, 128), np.float32)
    tri[np.arange(128)[:, None] > np.arange(128)[None, :]] = NEG
    w2 = [np.zeros((128, 896), f16) for _ in range(2)]
    for j in range(2):
        w2[j][:, 0:512] = wq
        w2[j][:, 512:640] = tri          # diag block of preload pair
        w2[j][:, 640:768] = NEG if j == 0 else 0.0  # past-diag block
        w2[j][:, 768:896] = np.eye(128, dtype=f16)

    in_maps = []
    for core in range(8):
        b, j = divmod(core, 2)
        xT = x[b].T.astype(f16)          # [1024, 2048]
        if j == 1:
            # swap adjacent 128-col blocks so q-cols sit at even positions
            xT = xT.reshape(1024, 8, 2, 128)[:, :, ::-1, :].reshape(1024, 2048)
        in_maps.append({
            "xt": np.ascontiguousarray(xT).reshape(8, 128, T),
            "wkv": wkv,
            "w2": w2[j],
        })
    return in_maps


def kernel(x, Wq, Wk, Wv):
    from concourse.bass_utils import run_bass_kernel_spmd

    x = np.asarray(x, dtype=np.float32)
    Wq = np.asarray(Wq, dtype=np.float32)
    Wk = np.asarray(Wk, dtype=np.float32)
    Wv = np.asarray(Wv, dtype=np.float32)

    if "nc" not in _CACHE:
        _CACHE["nc"] = _build()
    nc = _CACHE["nc"]

    in_maps = _host_inputs(x, Wq, Wk, Wv)
    res = run_bass_kernel_spmd(nc, in_maps, core_ids=list(range(8)))
    out = np.empty((B, T, DK), dtype=np.float32)
    for core in range(8):
        b, j = divmod(core, 2)
        yloc = res.results[core]["y"]    # [128, 512]
        for i in range(8):
            g = 2 * i + j
            out[b, 128 * g:128 * (g + 1), :] = yloc[:, 64 * i:64 * (i + 1)]
    return out
